# revision 11
# baseline (speedup 1.0000x reference)
"""Trainium2 Bass kernel for nn_GeneSetPlaceholderAggregator.

Computes out[b,s,d] = sum_g x[b,g,d] * W[s,g]  (einsum 'bgd,sg->bsd')
with B=64, G=20000, D=16, S=128.

Strategy:
- Shard the contraction axis G across 8 cores (2500 genes each, as 20
  chunks of K=125 partitions -- no padding).  Each core computes a full
  partial output [S=128, B*D=1024] via PSUM-accumulated matmuls
  (contraction on the partition dim); the host sums the 8 partials.
- bf16 inputs and outputs: the kernel is DMA-bound at ~420 GB/s/core, so
  halving the bytes halves the stream time.  fp32 PSUM accumulation keeps
  the 20000-term dot products accurate (fro rel err ~2e-3 vs 2e-2 gate).
- Host packs per-gene rows [x | W] partition-major ([125, chunk, row]) so
  every DMA descriptor is a contiguous run per partition; group DMAs are
  sized so compute starts early and the final group's matmul tail is short.
- Output copies PSUM->SBUF run on Scalar and Vector engines in parallel.
"""

import ml_dtypes
import numpy as np

import concourse.mybir as mybir
from concourse import bass
from concourse.bacc import Bacc
from concourse.bass_utils import run_bass_kernel_spmd
from concourse.tile import TileContext

B, G, D, S = 64, 20000, 16, 128
N_CORES = 8
K = 128                        # contraction tile = partition dim; 125 would
                               # avoid padding but factorizes 5x25 in the DGE
                               # partition split -> only 5 DMA engines (3.2x
                               # less bandwidth). 128 = 16x8 -> all 16 engines.
N_CHUNKS = 20                  # chunks per core
G_LOC = K * N_CHUNKS           # 2560 genes per core (zero-padded from 2500)
G_PAD = G_LOC * N_CORES        # 20480
BD = B * D                     # 1024
ROW = BD + S                   # 1152: [x row | w row] per gene
FREE = 512                     # fp32 free dim per PSUM bank / matmul
N_FREE = BD // FREE            # 2
# DMA group sizes (chunks). In-order completion per group semaphore; small
# first group lets matmuls start early, small last group keeps the tail
# after the final byte short (2 matmuls).
GROUP_SIZES = [2, 2, 3, 3, 4, 3, 2, 1]
N_GROUPS = len(GROUP_SIZES)

MM_DT = mybir.dt.bfloat16
NP_BF16 = ml_dtypes.bfloat16


def build_nc() -> bass.Bass:
    nc = Bacc("TRN2", target_bir_lowering=False, enable_partition_id=False)

    # xp[p, c*ROW + f] = packed row of gene (chunk c, partition p)
    xp_d = nc.declare_dram_parameter(
        "xp", [K, N_CHUNKS * ROW], MM_DT, isOutput=False
    )
    out = nc.declare_dram_parameter("out", [S, BD], MM_DT, isOutput=True)

    with TileContext(nc) as tc:
        with (
            tc.tile_pool(name="gp", bufs=1) as gp,
            tc.tile_pool(name="op", bufs=2) as op,
            tc.tile_pool(name="ps", bufs=N_FREE, space="PSUM") as ps,
        ):
            psums = [
                ps.tile([S, FREE], mybir.dt.float32, name=f"psum{j}")
                for j in range(N_FREE)
            ]
            tiles = []
            start_chunk = []
            c0 = 0
            # Alternate trigger issue across both HWDGE engines (SP +
            # Activation): each DMA_DIRECT2D takes ~700ns to issue, so
            # serializing all 8 on Sync delays the later groups' stream
            # start.  The PSUM->SBUF copies run on Vector/GpSimd so Scalar
            # stays free of ACT_TABLE_LOAD.
            for g, sz in enumerate(GROUP_SIZES):
                g_t = gp.tile([K, sz * ROW], MM_DT, name=f"grp{g}", tag=f"grp{g}")
                (nc.sync if g % 2 == 0 else nc.gpsimd).dma_start(
                    out=g_t[:], in_=xp_d[:, c0 * ROW:(c0 + sz) * ROW]
                )
                tiles.append(g_t)
                start_chunk.append(c0)
                c0 += sz
            for g, sz in enumerate(GROUP_SIZES):
                for l in range(sz):
                    c = start_chunk[g] + l
                    base = l * ROW
                    for j in range(N_FREE):
                        nc.tensor.matmul(
                            psums[j][:],
                            lhsT=tiles[g][:, base + BD:base + ROW],
                            rhs=tiles[g][:, base + j * FREE:base + (j + 1) * FREE],
                            start=(c == 0),
                            stop=(c == N_CHUNKS - 1),
                        )
            # Both PSUM->SBUF copies run in parallel (Scalar + Vector; only
            # those engines can read PSUM) into one tile, so a single output
            # DMA (2048B/partition packets) needs only one trigger.
            o_t = op.tile([S, BD], MM_DT)
            nc.scalar.copy(out=o_t[:, 0:FREE], in_=psums[0][:])
            nc.vector.tensor_copy(out=o_t[:, FREE:BD], in_=psums[1][:])
            nc.sync.dma_start(out=out[:, :], in_=o_t[:])
    nc.compile()
    return nc


_CACHE: dict = {}


def _get_nc() -> bass.Bass:
    if "nc" not in _CACHE:
        _CACHE["nc"] = build_nc()
    return _CACHE["nc"]


def _shard_inputs(x: np.ndarray, W: np.ndarray) -> list[dict[str, np.ndarray]]:
    # Packed per-gene rows [x[:, g, :].ravel() | W[:, g]] -> XW [G_PAD, ROW],
    # bf16, zero rows beyond G.  Then partition-major per core:
    # XP[i, p, c, :] = XW[i*G_LOC + c*K + p, :]
    XW = np.zeros((G_PAD, ROW), dtype=NP_BF16)
    XW[:G, :BD] = x.transpose(1, 0, 2).reshape(G, BD).astype(NP_BF16)
    XW[:G, BD:] = W.T.astype(NP_BF16)
    XP = np.ascontiguousarray(
        XW.reshape(N_CORES, N_CHUNKS, K, ROW).transpose(0, 2, 1, 3)
    ).reshape(N_CORES, K, N_CHUNKS * ROW)
    return [{"xp": XP[i]} for i in range(N_CORES)]


def run(x: np.ndarray, W: np.ndarray, **spmd_kwargs):
    nc = _get_nc()
    in_maps = _shard_inputs(x, W)
    res = run_bass_kernel_spmd(nc, in_maps, list(range(N_CORES)), **spmd_kwargs)
    partial = np.zeros((S, BD), dtype=np.float64)
    for r in res.results:
        partial += np.asarray(r["out"]).astype(np.float64)
    out = partial.astype(np.float32).reshape(S, B, D).transpose(1, 0, 2)
    return np.ascontiguousarray(out), res


def kernel(x: np.ndarray, W: np.ndarray) -> np.ndarray:
    out, _ = run(x, W)
    return out


# revision 12
# speedup vs baseline: 1.0204x; 1.0204x over previous
"""Trainium2 Bass kernel for nn_GeneSetPlaceholderAggregator.

Computes out[b,s,d] = sum_g x[b,g,d] * W[s,g]  (einsum 'bgd,sg->bsd')
with B=64, G=20000, D=16, S=128.

Strategy:
- Shard the contraction axis G across 8 cores (2500 genes each, as 20
  chunks of K=125 partitions -- no padding).  Each core computes a full
  partial output [S=128, B*D=1024] via PSUM-accumulated matmuls
  (contraction on the partition dim); the host sums the 8 partials.
- bf16 inputs and outputs: the kernel is DMA-bound at ~420 GB/s/core, so
  halving the bytes halves the stream time.  fp32 PSUM accumulation keeps
  the 20000-term dot products accurate (fro rel err ~2e-3 vs 2e-2 gate).
- Host packs per-gene rows [x | W] partition-major ([125, chunk, row]) so
  every DMA descriptor is a contiguous run per partition; group DMAs are
  sized so compute starts early and the final group's matmul tail is short.
- Output copies PSUM->SBUF run on Scalar and Vector engines in parallel.
"""

import ml_dtypes
import numpy as np

import concourse.mybir as mybir
from concourse import bass
from concourse.bacc import Bacc
from concourse.bass_utils import run_bass_kernel_spmd
from concourse.tile import TileContext

B, G, D, S = 64, 20000, 16, 128
N_CORES = 8
K = 128                        # contraction tile = partition dim; 125 would
                               # avoid padding but factorizes 5x25 in the DGE
                               # partition split -> only 5 DMA engines (3.2x
                               # less bandwidth). 128 = 16x8 -> all 16 engines.
N_CHUNKS = 20                  # chunks per core
G_LOC = K * N_CHUNKS           # 2560 genes per core (zero-padded from 2500)
G_PAD = G_LOC * N_CORES        # 20480
BD = B * D                     # 1024
ROW = BD + S                   # 1152: [x row | w row] per gene
FREE = 512                     # fp32 free dim per PSUM bank / matmul
N_FREE = BD // FREE            # 2
# DMA group sizes (chunks). In-order completion per group semaphore; small
# first group lets matmuls start early, small last group keeps the tail
# after the final byte short (2 matmuls).
GROUP_SIZES = [2, 2, 3, 3, 4, 3, 2, 1]
N_GROUPS = len(GROUP_SIZES)

MM_DT = mybir.dt.bfloat16
NP_BF16 = ml_dtypes.bfloat16


def build_nc() -> bass.Bass:
    nc = Bacc("TRN2", target_bir_lowering=False, enable_partition_id=False)

    # xp[p, c*ROW + f] = packed row of gene (chunk c, partition p)
    xp_d = nc.declare_dram_parameter(
        "xp", [K, N_CHUNKS * ROW], MM_DT, isOutput=False
    )
    out = nc.declare_dram_parameter("out", [S, BD], MM_DT, isOutput=True)

    with TileContext(nc) as tc:
        with (
            tc.tile_pool(name="gp", bufs=1) as gp,
            tc.tile_pool(name="op", bufs=2) as op,
            tc.tile_pool(name="ps", bufs=N_FREE, space="PSUM") as ps,
        ):
            psums = [
                ps.tile([S, FREE], mybir.dt.float32, name=f"psum{j}")
                for j in range(N_FREE)
            ]
            tiles = []
            start_chunk = []
            c0 = 0
            # Alternate trigger issue across both HWDGE engines (SP +
            # Activation): each DMA_DIRECT2D takes ~700ns to issue, so
            # serializing all 8 on Sync delays the later groups' stream
            # start.  The PSUM->SBUF copies run on Vector/GpSimd so Scalar
            # stays free of ACT_TABLE_LOAD.
            for g, sz in enumerate(GROUP_SIZES):
                g_t = gp.tile([K, sz * ROW], MM_DT, name=f"grp{g}", tag=f"grp{g}")
                (nc.sync if g % 2 == 0 else nc.scalar).dma_start(
                    out=g_t[:], in_=xp_d[:, c0 * ROW:(c0 + sz) * ROW]
                )
                tiles.append(g_t)
                start_chunk.append(c0)
                c0 += sz
            for g, sz in enumerate(GROUP_SIZES):
                for l in range(sz):
                    c = start_chunk[g] + l
                    base = l * ROW
                    for j in range(N_FREE):
                        nc.tensor.matmul(
                            psums[j][:],
                            lhsT=tiles[g][:, base + BD:base + ROW],
                            rhs=tiles[g][:, base + j * FREE:base + (j + 1) * FREE],
                            start=(c == 0),
                            stop=(c == N_CHUNKS - 1),
                        )
            # Both PSUM->SBUF copies run in parallel (Scalar + Vector; only
            # those engines can read PSUM) into one tile, so a single output
            # DMA (2048B/partition packets) needs only one trigger.
            o_t = op.tile([S, BD], MM_DT)
            nc.scalar.copy(out=o_t[:, 0:FREE], in_=psums[0][:])
            nc.vector.tensor_copy(out=o_t[:, FREE:BD], in_=psums[1][:])
            nc.sync.dma_start(out=out[:, :], in_=o_t[:])
    nc.compile()
    return nc


_CACHE: dict = {}


def _get_nc() -> bass.Bass:
    if "nc" not in _CACHE:
        _CACHE["nc"] = build_nc()
    return _CACHE["nc"]


def _shard_inputs(x: np.ndarray, W: np.ndarray) -> list[dict[str, np.ndarray]]:
    # Packed per-gene rows [x[:, g, :].ravel() | W[:, g]] -> XW [G_PAD, ROW],
    # bf16, zero rows beyond G.  Then partition-major per core:
    # XP[i, p, c, :] = XW[i*G_LOC + c*K + p, :]
    XW = np.zeros((G_PAD, ROW), dtype=NP_BF16)
    XW[:G, :BD] = x.transpose(1, 0, 2).reshape(G, BD).astype(NP_BF16)
    XW[:G, BD:] = W.T.astype(NP_BF16)
    XP = np.ascontiguousarray(
        XW.reshape(N_CORES, N_CHUNKS, K, ROW).transpose(0, 2, 1, 3)
    ).reshape(N_CORES, K, N_CHUNKS * ROW)
    return [{"xp": XP[i]} for i in range(N_CORES)]


def run(x: np.ndarray, W: np.ndarray, **spmd_kwargs):
    nc = _get_nc()
    in_maps = _shard_inputs(x, W)
    res = run_bass_kernel_spmd(nc, in_maps, list(range(N_CORES)), **spmd_kwargs)
    partial = np.zeros((S, BD), dtype=np.float64)
    for r in res.results:
        partial += np.asarray(r["out"]).astype(np.float64)
    out = partial.astype(np.float32).reshape(S, B, D).transpose(1, 0, 2)
    return np.ascontiguousarray(out), res


def kernel(x: np.ndarray, W: np.ndarray) -> np.ndarray:
    out, _ = run(x, W)
    return out


# revision 16
# speedup vs baseline: 1.1604x; 1.1371x over previous
"""Trainium2 Bass kernel for nn_GeneSetPlaceholderAggregator.

Computes out[b,s,d] = sum_g x[b,g,d] * W[s,g]  (einsum 'bgd,sg->bsd')
with B=64, G=20000, D=16, S=128.

Strategy:
- Shard the contraction axis G across 8 cores (2500 genes each, as 20
  chunks of K=125 partitions -- no padding).  Each core computes a full
  partial output [S=128, B*D=1024] via PSUM-accumulated matmuls
  (contraction on the partition dim); the host sums the 8 partials.
- bf16 inputs and outputs: the kernel is DMA-bound at ~420 GB/s/core, so
  halving the bytes halves the stream time.  fp32 PSUM accumulation keeps
  the 20000-term dot products accurate (fro rel err ~2e-3 vs 2e-2 gate).
- Host packs per-gene rows [x | W] partition-major ([125, chunk, row]) so
  every DMA descriptor is a contiguous run per partition; group DMAs are
  sized so compute starts early and the final group's matmul tail is short.
- Output copies PSUM->SBUF run on Scalar and Vector engines in parallel.
"""

import ml_dtypes
import numpy as np

import concourse.mybir as mybir
from concourse import bass
from concourse.bacc import Bacc
from concourse.bass_utils import run_bass_kernel_spmd
from concourse.tile import TileContext

B, G, D, S = 64, 20000, 16, 128
N_CORES = 8
K = 128                        # contraction tile = partition dim; 125 would
                               # avoid padding but factorizes 5x25 in the DGE
                               # partition split -> only 5 DMA engines (3.2x
                               # less bandwidth). 128 = 16x8 -> all 16 engines.
N_CHUNKS = 20                  # chunks per core
G_LOC = K * N_CHUNKS           # 2560 genes per core (zero-padded from 2500)
G_PAD = G_LOC * N_CORES        # 20480
BD = B * D                     # 1024
ROW = BD + S                   # 1152: [x row | w row] per gene
FREE = 512                     # fp32 free dim per PSUM bank / matmul
N_FREE = BD // FREE            # 2
# DMA group sizes (chunks). In-order completion per group semaphore; small
# first group lets matmuls start early, small last group keeps the tail
# after the final byte short (2 matmuls).
GROUP_SIZES = [1, 2, 3, 4, 4, 3, 2, 1]
N_GROUPS = len(GROUP_SIZES)

MM_DT = mybir.dt.bfloat16
NP_BF16 = ml_dtypes.bfloat16


def build_nc() -> bass.Bass:
    nc = Bacc("TRN2", target_bir_lowering=False, enable_partition_id=False)

    # xp[p, c*ROW + f] = packed row of gene (chunk c, partition p)
    xp_d = nc.declare_dram_parameter(
        "xp", [K, N_CHUNKS * ROW], MM_DT, isOutput=False
    )
    out = nc.declare_dram_parameter("out", [S, BD], MM_DT, isOutput=True)

    with TileContext(nc) as tc:
        with (
            tc.tile_pool(name="gp", bufs=1) as gp,
            tc.tile_pool(name="op", bufs=2) as op,
            tc.tile_pool(name="ps", bufs=N_FREE, space="PSUM") as ps,
        ):
            psums = [
                ps.tile([S, FREE], mybir.dt.float32, name=f"psum{j}")
                for j in range(N_FREE)
            ]
            tiles = []
            start_chunk = []
            c0 = 0
            # All input triggers go on the single Sync HWDGE queue: one
            # queue drains groups in order, so each group's completion
            # semaphore fires right as its burst ends (splitting across two
            # queues interleaves packets and delays every group's
            # completion, stalling the matmul chain).
            for g, sz in enumerate(GROUP_SIZES):
                g_t = gp.tile([K, sz * ROW], MM_DT, name=f"grp{g}", tag=f"grp{g}")
                nc.sync.dma_start(
                    out=g_t[:], in_=xp_d[:, c0 * ROW:(c0 + sz) * ROW]
                )
                tiles.append(g_t)
                start_chunk.append(c0)
                c0 += sz
            for g, sz in enumerate(GROUP_SIZES):
                for l in range(sz):
                    c = start_chunk[g] + l
                    base = l * ROW
                    for j in range(N_FREE):
                        nc.tensor.matmul(
                            psums[j][:],
                            lhsT=tiles[g][:, base + BD:base + ROW],
                            rhs=tiles[g][:, base + j * FREE:base + (j + 1) * FREE],
                            start=(c == 0),
                            stop=(c == N_CHUNKS - 1),
                        )
            # Both PSUM->SBUF copies run in parallel (Scalar + Vector; only
            # those engines can read PSUM) into one tile, so a single output
            # DMA (2048B/partition packets) needs only one trigger.
            o_t = op.tile([S, BD], MM_DT)
            nc.scalar.copy(out=o_t[:, 0:FREE], in_=psums[0][:])
            nc.vector.tensor_copy(out=o_t[:, FREE:BD], in_=psums[1][:])
            # Output trigger on the Activation HWDGE queue so it doesn't
            # queue behind the input groups on Sync.
            nc.scalar.dma_start(out=out[:, :], in_=o_t[:])
    nc.compile()
    return nc


_CACHE: dict = {}


def _get_nc() -> bass.Bass:
    if "nc" not in _CACHE:
        _CACHE["nc"] = build_nc()
    return _CACHE["nc"]


def _shard_inputs(x: np.ndarray, W: np.ndarray) -> list[dict[str, np.ndarray]]:
    # Packed per-gene rows [x[:, g, :].ravel() | W[:, g]] -> XW [G_PAD, ROW],
    # bf16, zero rows beyond G.  Then partition-major per core:
    # XP[i, p, c, :] = XW[i*G_LOC + c*K + p, :]
    XW = np.zeros((G_PAD, ROW), dtype=NP_BF16)
    XW[:G, :BD] = x.transpose(1, 0, 2).reshape(G, BD).astype(NP_BF16)
    XW[:G, BD:] = W.T.astype(NP_BF16)
    XP = np.ascontiguousarray(
        XW.reshape(N_CORES, N_CHUNKS, K, ROW).transpose(0, 2, 1, 3)
    ).reshape(N_CORES, K, N_CHUNKS * ROW)
    return [{"xp": XP[i]} for i in range(N_CORES)]


def run(x: np.ndarray, W: np.ndarray, **spmd_kwargs):
    nc = _get_nc()
    in_maps = _shard_inputs(x, W)
    res = run_bass_kernel_spmd(nc, in_maps, list(range(N_CORES)), **spmd_kwargs)
    partial = np.zeros((S, BD), dtype=np.float64)
    for r in res.results:
        partial += np.asarray(r["out"]).astype(np.float64)
    out = partial.astype(np.float32).reshape(S, B, D).transpose(1, 0, 2)
    return np.ascontiguousarray(out), res


def kernel(x: np.ndarray, W: np.ndarray) -> np.ndarray:
    out, _ = run(x, W)
    return out


# revision 21
# speedup vs baseline: 1.2032x; 1.0369x over previous
"""Trainium2 Bass kernel for nn_GeneSetPlaceholderAggregator.

Computes out[b,s,d] = sum_g x[b,g,d] * W[s,g]  (einsum 'bgd,sg->bsd')
with B=64, G=20000, D=16, S=128.

Strategy:
- Shard the contraction axis G across 8 cores (2500 genes each, as 20
  chunks of K=125 partitions -- no padding).  Each core computes a full
  partial output [S=128, B*D=1024] via PSUM-accumulated matmuls
  (contraction on the partition dim); the host sums the 8 partials.
- bf16 inputs and outputs: the kernel is DMA-bound at ~420 GB/s/core, so
  halving the bytes halves the stream time.  fp32 PSUM accumulation keeps
  the 20000-term dot products accurate (fro rel err ~2e-3 vs 2e-2 gate).
- Host packs per-gene rows [x | W] partition-major ([125, chunk, row]) so
  every DMA descriptor is a contiguous run per partition; group DMAs are
  sized so compute starts early and the final group's matmul tail is short.
- Output copies PSUM->SBUF run on Scalar and Vector engines in parallel.
"""

import ml_dtypes
import numpy as np

import concourse.mybir as mybir
from concourse import bass
from concourse.bacc import Bacc
from concourse.bass_utils import run_bass_kernel_spmd
from concourse.tile import TileContext

B, G, D, S = 64, 20000, 16, 128
N_CORES = 8
K = 128                        # contraction tile = partition dim; 125 would
                               # avoid padding but factorizes 5x25 in the DGE
                               # partition split -> only 5 DMA engines (3.2x
                               # less bandwidth). 128 = 16x8 -> all 16 engines.
N_CHUNKS = 20                  # chunks per core
G_LOC = K * N_CHUNKS           # 2560 genes per core (zero-padded from 2500)
G_PAD = G_LOC * N_CORES        # 20480
BD = B * D                     # 1024
ROW = BD + S                   # 1152: [x row | w row] per gene
FREE = 512                     # fp32 free dim per PSUM bank / matmul
N_FREE = BD // FREE            # 2
# DMA group sizes (chunks). In-order completion per group semaphore.  Big
# front groups keep the descriptor-generator ring deep during the stream
# ramp (small early groups cause throughput dips at group boundaries);
# small tail groups keep the matmul tail after the final byte short.  The
# matmul chain has slack at the start, so a late first-group completion is
# free.
GROUP_SIZES = [4, 4, 3, 3, 2, 2, 1, 1]
N_GROUPS = len(GROUP_SIZES)

MM_DT = mybir.dt.bfloat16
NP_BF16 = ml_dtypes.bfloat16


def build_nc() -> bass.Bass:
    nc = Bacc("TRN2", target_bir_lowering=False, enable_partition_id=False)

    # xp[p, c*ROW + f] = packed row of gene (chunk c, partition p)
    xp_d = nc.declare_dram_parameter(
        "xp", [K, N_CHUNKS * ROW], MM_DT, isOutput=False
    )
    out = nc.declare_dram_parameter("out", [S, BD], MM_DT, isOutput=True)

    with TileContext(nc) as tc:
        with (
            tc.tile_pool(name="gp", bufs=1) as gp,
            tc.tile_pool(name="op", bufs=2) as op,
            tc.tile_pool(name="ps", bufs=1, space="PSUM") as ps,
        ):
            # One PSUM tile spanning N_FREE adjacent banks; each matmul
            # accumulates into its own 512-col bank slice, and a single DVE
            # cast drains the whole thing at the end.
            psum_t = ps.tile([S, BD], mybir.dt.float32, name="psum")
            psums = [psum_t[:, j * FREE:(j + 1) * FREE] for j in range(N_FREE)]
            tiles = []
            start_chunk = []
            c0 = 0
            # All input triggers go on the single Sync HWDGE queue: one
            # queue drains groups in order, so each group's completion
            # semaphore fires right as its burst ends (splitting across two
            # queues interleaves packets and delays every group's
            # completion, stalling the matmul chain).
            for g, sz in enumerate(GROUP_SIZES):
                g_t = gp.tile([K, sz * ROW], MM_DT, name=f"grp{g}", tag=f"grp{g}")
                nc.sync.dma_start(
                    out=g_t[:], in_=xp_d[:, c0 * ROW:(c0 + sz) * ROW]
                )
                tiles.append(g_t)
                start_chunk.append(c0)
                c0 += sz
            for g, sz in enumerate(GROUP_SIZES):
                for l in range(sz):
                    c = start_chunk[g] + l
                    base = l * ROW
                    for j in range(N_FREE):
                        nc.tensor.matmul(
                            psums[j],
                            lhsT=tiles[g][:, base + BD:base + ROW],
                            rhs=tiles[g][:, base + j * FREE:base + (j + 1) * FREE],
                            start=(c == 0),
                            stop=(c == N_CHUNKS - 1),
                        )
            # Single DVE cast PSUM->SBUF bf16, then one output DMA
            # (2048B/partition packets) triggered from the Activation HWDGE
            # queue so it doesn't queue behind the input groups on Sync.
            o_t = op.tile([S, BD], MM_DT)
            nc.vector.tensor_copy(out=o_t[:], in_=psum_t[:])
            nc.scalar.dma_start(out=out[:, :], in_=o_t[:])
    nc.compile()
    return nc


_CACHE: dict = {}


def _get_nc() -> bass.Bass:
    if "nc" not in _CACHE:
        _CACHE["nc"] = build_nc()
    return _CACHE["nc"]


def _shard_inputs(x: np.ndarray, W: np.ndarray) -> list[dict[str, np.ndarray]]:
    # Packed per-gene rows [x[:, g, :].ravel() | W[:, g]] -> XW [G_PAD, ROW],
    # bf16, zero rows beyond G.  Then partition-major per core:
    # XP[i, p, c, :] = XW[i*G_LOC + c*K + p, :]
    XW = np.zeros((G_PAD, ROW), dtype=NP_BF16)
    XW[:G, :BD] = x.transpose(1, 0, 2).reshape(G, BD).astype(NP_BF16)
    XW[:G, BD:] = W.T.astype(NP_BF16)
    XP = np.ascontiguousarray(
        XW.reshape(N_CORES, N_CHUNKS, K, ROW).transpose(0, 2, 1, 3)
    ).reshape(N_CORES, K, N_CHUNKS * ROW)
    return [{"xp": XP[i]} for i in range(N_CORES)]


def run(x: np.ndarray, W: np.ndarray, **spmd_kwargs):
    nc = _get_nc()
    in_maps = _shard_inputs(x, W)
    res = run_bass_kernel_spmd(nc, in_maps, list(range(N_CORES)), **spmd_kwargs)
    partial = np.zeros((S, BD), dtype=np.float64)
    for r in res.results:
        partial += np.asarray(r["out"]).astype(np.float64)
    out = partial.astype(np.float32).reshape(S, B, D).transpose(1, 0, 2)
    return np.ascontiguousarray(out), res


def kernel(x: np.ndarray, W: np.ndarray) -> np.ndarray:
    out, _ = run(x, W)
    return out
